# revision 11
# baseline (speedup 1.0000x reference)
"""TRN2 Bass kernel for nn_Attention_86260123173325.

Single-head attention over N=4096 tokens, feature dim HW=4096:
  q, k, v = x[:,0], x[:,1], x[:,2] reshaped to [4096, 4096]
  out = softmax(0.5 * q @ k.T) @ v

Sharding: q rows split across 8 cores (512 rows each); k, v replicated.

Per-core algorithm (matmuls in f32r = TF32-like, 4x the fp32 PE rate):
  - PE-transpose q shard -> qT[d, i]
  - Phase 1, per 128-row k block j: PE-transpose k block -> kT,
    R^T[j,:] = k_j @ q^T via 32 accumulated f32r matmuls (R = raw q.k dots,
    dp = 0.5*R). Keep R^T in SBUF (fp32), and accumulate a row statistic
    W_i = sum_j exp(0.2*dp_ij) via exp on ACT + a ones^T @ E_t matmul
    (out is a [1,512] PSUM row, full-rate N=512).
  - shift_i = 5*ln(W_i) = LSE_{t=0.2} / t >= rowmax_i, and
    shift_i <= rowmax_i + ln(4096)/0.2. Since any per-row shift cancels in
    the final normalization, exp(dp - shift) is exact softmax numerator and
    can never overflow (dp - shift <= 0) nor lose relevant terms
    (relevant j's stay >= e^-67). Works for any data with |dp| < ~440.
  - Pass 2: eT = exp(0.5*R - shift) written in-place over R^T (f32r view).
  - rowsum via E^T.T @ ones (N=1 matmuls); rinv = 1/rowsum
  - Phase 2: O = (E @ v) * rinv, accumulating over j blocks in PSUM.
"""
import sys

sys.path.insert(0, "/opt/trn_rl_repo")

import numpy as np

import concourse.bass as bass
import concourse.tile as tile
from concourse import bacc, mybir
from concourse.bass_utils import run_bass_kernel_spmd
from concourse.masks import make_identity

F32 = mybir.dt.float32
F32R = mybir.dt.float32r
EXP = mybir.ActivationFunctionType.Exp
LN = mybir.ActivationFunctionType.Ln

N_CORES = 8
N = 4096          # tokens (keys)
D = 4096          # feature dim (H*W)
M = N // N_CORES  # q rows per core = 512
NJ = N // 128     # 32 key blocks
ND = D // 128     # 32 feature blocks
NI = M // 128     # 4 q-row blocks per core
NDT = D // 512    # 8 output column tiles
T_STAT = 0.2      # stage-1 temperature: exp(t*dp) = exp(0.1*R)


def _build_nc():
    nc = bacc.Bacc(None, target_bir_lowering=False, debug=False)

    q_dram = nc.dram_tensor("q", [M, D], F32R, kind="ExternalInput")
    k_dram = nc.dram_tensor("k", [N, D], F32R, kind="ExternalInput")
    v_dram = nc.dram_tensor("v", [N, D], F32R, kind="ExternalInput")
    o_dram = nc.dram_tensor("o", [M, D], F32, kind="ExternalOutput")
    sh2_dram = nc.dram_tensor("sh2_scratch", [1, M], F32)

    with tile.TileContext(nc) as tc:
        with tc.tile_pool(name="persist", bufs=1) as persist:
            # R^T / E^T storage, [j-within-block, j-block, i]; E^T overwrites
            # R^T in place (pass 2), read as f32r via bitcast.
            s_sb = persist.tile([128, NJ, M], F32)

            idf = persist.tile([128, 128], F32, tag="idf")
            make_identity(nc, idf[:])
            idr = persist.tile([128, 128], F32R, tag="idr")
            nc.vector.tensor_copy(idr[:], idf[:])

            # [128, 2]: f32r matmuls require even free sizes on all operands
            ones_f = persist.tile([128, 2], F32, tag="ones_f")
            nc.vector.memset(ones_f[:], 1.0)
            ones_r = persist.tile([128, 2], F32R, tag="ones_r")
            nc.vector.tensor_copy(ones_r[:], ones_f[:])

            zero_b = persist.tile([128, 1], F32, tag="zero_b")
            nc.vector.memset(zero_b[:], 0.0)

            # stage-1 exp bias: keeps W = sum exp(0.2*dp - 40) well under
            # ~2^64, where the HW exp/f32r-matmul/ln chain was observed to
            # produce junk (rows with rowmax>232 went NaN without it)
            stat_b = persist.tile([128, 1], F32, tag="stat_b")
            nc.vector.memset(stat_b[:], -40.0)

            sh2_bc = persist.tile([128, M], F32, tag="sh2_bc")
            rsum = persist.tile([128, NI], F32, tag="rsum")
            rinv = persist.tile([128, NI], F32, tag="rinv")

            # ---------------- phase 1: R^T blocks + W stats ----------------
            with (
                tc.tile_pool(name="qsrc", bufs=2) as qpool,
                tc.tile_pool(name="qT", bufs=1) as qTpool,
                tc.tile_pool(name="ksrc", bufs=3) as kpool,
                tc.tile_pool(name="kT", bufs=5) as kTpool,
                tc.tile_pool(name="ett", bufs=2) as etpool,
                tc.tile_pool(name="psA", bufs=3, space="PSUM") as psA,
                tc.tile_pool(name="psS", bufs=2, space="PSUM") as psS,
                tc.tile_pool(name="psW", bufs=1, space="PSUM") as psWp,
            ):
                # qT[p, dblk, i] = q[i, dblk*128+p]
                qT = qTpool.tile([128, ND, M], F32R)
                for ib in range(NI):
                    for half in range(2):
                        qsb = qpool.tile([128, D // 2], F32R, tag="qsrc",
                                         name=f"q{ib}_{half}")
                        nc.sync.dma_start(
                            out=qsb[:],
                            in_=q_dram[ib * 128:(ib + 1) * 128,
                                       half * 2048:(half + 1) * 2048],
                        )
                        for b in range(ND // 8):
                            pt = psA.tile([128, 512], F32R, tag="tp",
                                          name=f"qpt{ib}_{half}_{b}")
                            for t in range(4):
                                dloc = 4 * b + t       # within half, 0..15
                                dblk = half * 16 + dloc
                                nc.tensor.transpose(
                                    pt[:, t * 128:(t + 1) * 128],
                                    qsb[:, dloc * 128:(dloc + 1) * 128],
                                    idr[:],
                                )
                            d0 = half * 16 + 4 * b
                            nc.vector.tensor_copy(
                                qT[:, d0:d0 + 4, ib * 128:(ib + 1) * 128],
                                pt[:].rearrange("p (t c) -> p t c", t=4),
                            )

                psW = psWp.tile([2, M], F32)
                # stream k blocks: transpose, matmul, stash R^T, W stat
                for j in range(NJ):
                    quarters = [
                        kpool.tile([128, D // 4], F32R, tag="ksrc",
                                   name=f"k{j}_{qi}")
                        for qi in range(4)
                    ]
                    for qi, kq in enumerate(quarters):
                        nc.sync.dma_start(
                            out=kq[:],
                            in_=k_dram[j * 128:(j + 1) * 128,
                                       qi * 1024:(qi + 1) * 1024],
                        )
                    # kT in 4 parts of 8 dblks each for finer PE interleave
                    kparts = []
                    for b in range(ND // 4):
                        if b % 2 == 0:
                            kTp = kTpool.tile([128, 8, 128], F32R, tag="kT",
                                              name=f"kT{j}_{b // 2}")
                            kparts.append(kTp)
                        kq = quarters[b // 2]
                        pt = psA.tile([128, 512], F32R, tag="tp",
                                      name=f"pt{j}_{b}")
                        for t in range(4):
                            dloc = (4 * b + t) % 8  # dblk within quarter
                            nc.tensor.transpose(
                                pt[:, t * 128:(t + 1) * 128],
                                kq[:, dloc * 128:(dloc + 1) * 128],
                                idr[:],
                            )
                        w0 = (b % 2) * 4
                        nc.vector.tensor_copy(
                            kparts[-1][:, w0:w0 + 4, :],
                            pt[:].rearrange("p (t c) -> p t c", t=4),
                        )
                    ps = psS.tile([128, M], F32, tag="S", name=f"ps{j}")
                    for dblk in range(ND):
                        nc.tensor.matmul(
                            ps[:],
                            kparts[dblk // 8][:, dblk % 8, :],
                            qT[:, dblk, :],
                            start=(dblk == 0),
                            stop=(dblk == ND - 1),
                        )
                    # stash raw scores R^T (fp32)
                    nc.vector.tensor_copy(s_sb[:, j, :], ps[:])
                    # W stat: exp(0.1*R) then ones^T @ E_t -> psW [1, M]
                    ett = etpool.tile([128, M], F32R, tag="ett", name=f"et{j}")
                    nc.scalar.activation(
                        out=ett[:], in_=ps[:], func=EXP,
                        bias=stat_b[:], scale=0.5 * T_STAT,
                    )
                    nc.tensor.matmul(
                        psW[:],
                        ones_r[:],
                        ett[:],
                        start=(j == 0),
                        stop=(j == NJ - 1),
                        skip_group_check=True,
                    )

                # shift row: sh2 = 2*shift = (2/t)*ln(W), broadcast to all partitions
                w_ln = persist.tile([1, M], F32, tag="w_ln")
                nc.scalar.activation(
                    out=w_ln[:], in_=psW[0:1, :], func=LN,
                    bias=zero_b[:1, :], scale=1.0,
                )
                sh2_row = persist.tile([1, M], F32, tag="sh2_row")
                # sh2 = (2/t)*(lnW' + 40) = 10*lnW' + 400
                nc.vector.tensor_scalar(
                    sh2_row[:], w_ln[:], 2.0 / T_STAT, 400.0,
                    mybir.AluOpType.mult, mybir.AluOpType.add,
                )
                # broadcast to all 128 partitions via a DRAM bounce
                # (SBUF source APs cannot have partition stride 0)
                nc.sync.dma_start(out=sh2_dram[:], in_=sh2_row[:])
                nc.gpsimd.dma_start(
                    out=sh2_bc[:], in_=sh2_dram[:].to_broadcast((128, M))
                )

            # ---------------- pass 2: eT = exp(0.5*R - shift) ------
            # (separate f32r tensor: the BIR verifier requires f32r matmul
            # inputs to be produced rounded, so no fp32-bitcast reuse; fits
            # because the qT pool has closed by now)
            eTstack = tc.tile_pool(name="eTp", bufs=1)
            eTpool = eTstack.__enter__()
            eT_t = eTpool.tile([128, NJ, M], F32R, name="eT_t")
            with tc.tile_pool(name="tmp", bufs=3) as tmpool:
                for j in range(NJ):
                    tmp = tmpool.tile([128, M], F32, tag="tmp", name=f"tmp{j}")
                    nc.vector.tensor_sub(tmp[:], s_sb[:, j, :], sh2_bc[:])
                    nc.scalar.activation(
                        out=eT_t[:, j, :], in_=tmp[:],
                        func=EXP, bias=zero_b[:], scale=0.5,
                    )

            def eT(j, i0, i1):
                return eT_t[:, j, i0:i1]

            # ---------------- rowsums ----------------
            with tc.tile_pool(name="psR", bufs=NI, space="PSUM") as psR:
                for ib in range(NI):
                    pr = psR.tile([128, 2], F32, tag="r", name=f"pr{ib}")
                    for j in range(NJ):
                        nc.tensor.matmul(
                            pr[:],
                            eT(j, ib * 128, (ib + 1) * 128),
                            ones_r[:],
                            start=(j == 0),
                            stop=(j == NJ - 1),
                        )
                    nc.vector.tensor_copy(rsum[:, ib:ib + 1], pr[:, 0:1])
                nc.vector.reciprocal(rinv[:], rsum[:])

            # ---------------- phase 2: O = (E @ v) * rinv ----------------
            with (
                tc.tile_pool(name="vsrc", bufs=4) as vpool,
                tc.tile_pool(name="osb", bufs=6) as opool,
                tc.tile_pool(name="psO", bufs=6, space="PSUM") as psO,
            ):
                for dt in range(NDT):
                    pos = [
                        psO.tile([128, 512], F32, tag="o", name=f"po{dt}_{ib}")
                        for ib in range(NI)
                    ]
                    for j in range(NJ):
                        vsb = vpool.tile([128, 512], F32R, tag="v",
                                         name=f"v{dt}_{j}")
                        nc.sync.dma_start(
                            out=vsb[:],
                            in_=v_dram[j * 128:(j + 1) * 128,
                                       dt * 512:(dt + 1) * 512],
                        )
                        for ib in range(NI):
                            nc.tensor.matmul(
                                pos[ib][:],
                                eT(j, ib * 128, (ib + 1) * 128),
                                vsb[:],
                                start=(j == 0),
                                stop=(j == NJ - 1),
                            )
                    for ib in range(NI):
                        osb = opool.tile([128, 512], F32, tag="osb",
                                         name=f"ob{dt}_{ib}")
                        nc.vector.tensor_scalar_mul(
                            osb[:], pos[ib][:], rinv[:, ib:ib + 1]
                        )
                        nc.sync.dma_start(
                            out=o_dram[ib * 128:(ib + 1) * 128,
                                       dt * 512:(dt + 1) * 512],
                            in_=osb[:],
                        )
            eTstack.__exit__(None, None, None)

    nc.compile()
    return nc


_NC_CACHE = None


def _get_nc():
    global _NC_CACHE
    if _NC_CACHE is None:
        _NC_CACHE = _build_nc()
    return _NC_CACHE


def _make_in_maps(x: np.ndarray) -> list:
    x = np.asarray(x)
    n, c, h, w = x.shape
    assert (n, c, h * w) == (N, 3, D), f"unexpected shape {x.shape}"
    xr = np.ascontiguousarray(x.reshape(n, c, h * w).transpose(1, 0, 2))
    q_full, k, v = xr[0], xr[1], xr[2]
    return [
        {
            "q": np.ascontiguousarray(q_full[core * M:(core + 1) * M]),
            "k": k,
            "v": v,
        }
        for core in range(N_CORES)
    ]


def kernel(x: np.ndarray) -> np.ndarray:
    nc = _get_nc()
    res = run_bass_kernel_spmd(nc, _make_in_maps(x), core_ids=list(range(N_CORES)))
    out = np.concatenate([r["o"] for r in res.results], axis=0)
    return out.astype(np.float32)
